# revision 31
# baseline (speedup 1.0000x reference)
"""Token-sharded Trainium2 Bass kernel for nn_MultiHeadAttention — NO
collectives (B=2, S=2048, E=1024, H=16, 8 cores).

Each core owns one 512-token slice of one batch and computes its full output
rows locally: Q projection for its tokens (all 16 heads), K/V projections
for ALL tokens (all heads), 16-head causal attention for its q-block, and
the full fc_out contraction. Causality is handled with per-core 0/1 bf16
masks multiplied into exp(scores) on DVE (2x bf16 rate); the softmax
denominator rides as a ones-column in V, so masked probabilities drop out
of both numerator and denominator. No inter-core communication of any kind
(the proxied collective path on this rig costs a flat ~700us/iteration).
"""

import numpy as np

N_CORES = 8
B, S, E, H = 2, 2048, 1024, 16
DK = E // H  # 64
QT = 512
NQT = S // QT  # 4
NPR = H // 2  # 8 head-pairs
WV65 = H * 65  # 1040
WQKV = E + E + WV65  # 3088
NKT = S // 128  # 16

_CACHE = {}


def _build(sim1=False, niter=1):
    import concourse.bacc as bacc
    import concourse.bass as bass
    import concourse.mybir as mybir
    import concourse.tile as tile

    f32 = mybir.dt.float32
    bf16 = mybir.dt.bfloat16

    nc = bacc.Bacc("TRN2", target_bir_lowering=False, debug=False,
                   num_devices=1 if sim1 else N_CORES)

    xq_d = nc.dram_tensor("xqTm", [E, QT], bf16, kind="ExternalInput")
    xk_d = nc.dram_tensor("xkT", [E, S], bf16, kind="ExternalInput")
    xv_d = nc.dram_tensor("xvT", [E, S], bf16, kind="ExternalInput")
    wqkv_d = nc.dram_tensor("wqkv", [E, WQKV], bf16, kind="ExternalInput")
    wo_d = nc.dram_tensor("wo", [E, E], bf16, kind="ExternalInput")
    bo_d = nc.dram_tensor("bo_t", [128, 8], f32, kind="ExternalInput")
    mask_d = nc.dram_tensor("mask01", [NKT, 128, QT], bf16,
                            kind="ExternalInput")
    out_d = nc.dram_tensor("outT", [E, QT], f32, kind="ExternalOutput")

    Exp = mybir.ActivationFunctionType.Exp
    Ident = mybir.ActivationFunctionType.Identity
    Mult = mybir.AluOpType.mult

    with tile.TileContext(nc) as tc:
        with (
            tc.tile_pool(name="const", bufs=1) as constp,
            tc.tile_pool(name="wop", bufs=1) as wop,
        ):
            for _it in range(niter):
              with (
                  tc.tile_pool(name=f"qkv{_it}", bufs=1) as qkvp,
                  tc.tile_pool(name=f"wgt{_it}", bufs=1) as wgtp,
                  tc.tile_pool(name=f"pps{_it}", bufs=2, space="PSUM") as ppsp,
                  tc.tile_pool(name=f"spool{_it}", bufs=2, space="PSUM") as spool,
                  tc.tile_pool(name=f"cpool{_it}", bufs=1, space="PSUM") as cpool,
                  tc.tile_pool(name=f"ppool{_it}", bufs=3) as ppool,
                  tc.tile_pool(name=f"rpool{_it}", bufs=2) as rpool,
                  tc.tile_pool(name=f"otp{_it}", bufs=3) as otp,
              ):
                  qT8 = qkvp.tile([128, NPR * QT], bf16, name=f"qT8_{_it}")
                  kT8 = [qkvp.tile([128, S], bf16, name=f"kT{pr}_{_it}")
                         for pr in range(NPR)]
                  vE = [qkvp.tile([128, WV65], bf16, name=f"vE{st}_{_it}")
                        for st in range(NKT)]
                  ctx_sb = qkvp.tile([128, NPR * QT], bf16,
                                     name=f"ctx{_it}")
                  mask_sb = constp.tile([128, NKT * QT], bf16,
                                        tag="mk", name=f"mk{_it}")
                  nc.gpsimd.dma_start(
                      mask_sb[:].rearrange("p (t q) -> p t q", t=NKT),
                      mask_d.ap().rearrange("t k q -> k t q"))
                  bo_sb = constp.tile([128, 8], f32, tag="bo", name=f"bo{_it}")
                  nc.gpsimd.dma_start(bo_sb[:], bo_d.ap())
                  wo_sb = wop.tile([128, 8 * E], bf16, tag="wo",
                                   name=f"wo{_it}")
                  nc.gpsimd.dma_start(
                      wo_sb[:].rearrange("p (t m) -> p t m", t=8),
                      wo_d.ap().rearrange("(t p) m -> p t m", p=128),
                  )

                  wk_sb = wgtp.tile([128, 8 * (E + WV65)], bf16,
                                    name=f"wkv{_it}")
                  wkv_v = wk_sb[:].rearrange("p (t m) -> p t m", t=8)
                  src_kv = (wqkv_d.ap()
                            .rearrange("(t p) m -> p t m", p=128)
                            [:, :, E:WQKV])
                  nc.scalar.dma_start(wkv_v, src_kv)

                  # ---- Q projection (own tokens, all heads) ----
                  with (
                      tc.tile_pool(name=f"wqp{_it}", bufs=1) as wqp,
                      tc.tile_pool(name=f"chq{_it}", bufs=1) as chqp,
                  ):
                      wq_sb = wqp.tile([128, 8 * E], bf16, name=f"wq{_it}")
                      wq_v = wq_sb[:].rearrange("p (t m) -> p t m", t=8)
                      nc.scalar.dma_start(
                          wq_v,
                          wqkv_d.ap().rearrange("(t p) m -> p t m", p=128)
                          [:, :, 0:E])
                      chq = chqp.tile([128, 8 * QT], bf16, name=f"chq{_it}")
                      chq_v = chq[:].rearrange("p (t q) -> p t q", t=8)
                      nc.sync.dma_start(
                          chq_v, xq_d.ap().rearrange("(t p) q -> p t q",
                                                     p=128))
                      for pr in range(NPR):
                          ps = ppsp.tile([128, QT], f32, tag="pp",
                                         name=f"pq{pr}_{_it}")
                          for kt in range(8):
                              nc.tensor.matmul(
                                  ps[:],
                                  wq_v[:, kt, 128 * pr:128 * pr + 128],
                                  chq[:, QT * kt:QT * kt + QT],
                                  start=(kt == 0), stop=(kt == 7),
                              )
                          nc.vector.tensor_copy(
                              qT8[:, QT * pr:QT * pr + QT], ps[:])

                  # ---- K/V projections (all tokens, all heads) ----
                  with tc.tile_pool(name=f"xt{_it}", bufs=4) as xtp:
                      for nt in range(NQT):
                          chs = []
                          for x_d, nm in ((xk_d, "k"), (xv_d, "v")):
                              t = xtp.tile([128, 8 * QT], bf16, tag="xt",
                                           name=f"ch{nm}{nt}_{_it}")
                              nc.sync.dma_start(
                                  t[:].rearrange("p (t q) -> p t q", t=8),
                                  x_d[:, QT * nt:QT * nt + QT]
                                  .rearrange("(t p) q -> p t q", p=128))
                              chs.append(t)
                          chk, chv = chs
                          for pr in range(NPR):
                              ps = ppsp.tile([128, QT], f32, tag="pp",
                                             name=f"pk{nt}_{pr}_{_it}")
                              for kt in range(8):
                                  nc.tensor.matmul(
                                      ps[:],
                                      wkv_v[:, kt, 128 * pr:128 * pr + 128],
                                      chk[:, QT * kt:QT * kt + QT],
                                      start=(kt == 0), stop=(kt == 7),
                                  )
                              nc.vector.tensor_copy(
                                  kT8[pr][:, QT * nt:QT * nt + QT], ps[:])
                          for sst in range(4):
                              st = 4 * nt + sst
                              for half in range(4):
                                  ps = ppsp.tile([128, 260], f32, tag="pp",
                                                 name=f"pv{st}_{half}_{_it}")
                                  for kt in range(8):
                                      nc.tensor.matmul(
                                          ps[:],
                                          chv[:, QT * kt + 128 * sst:
                                              QT * kt + 128 * sst + 128],
                                          wkv_v[:, kt,
                                                E + 260 * half:
                                                E + 260 * half + 260],
                                          start=(kt == 0), stop=(kt == 7),
                                      )
                                  nc.vector.tensor_copy(
                                      vE[st][:, 260 * half:260 * half + 260],
                                      ps[:])
                              # softmax-denominator ones columns
                              nc.gpsimd.memset(
                                  vE[st][:].rearrange("p (h c) -> p h c",
                                                      h=H)[:, :, 64:65],
                                  1.0)

                  # ---- attention: all 16 heads for own q-block ----
                  for pr in range(NPR):
                      ctxA = cpool.tile([65, QT], f32, tag="ctxA")
                      ctxB = cpool.tile([65, QT], f32, tag="ctxB")
                      for kt in range(NKT):
                          sS = spool.tile([128, 2 * QT], f32, tag="s")
                          sv = sS[:].rearrange("k (h q) -> k h q", h=2)
                          for h in range(2):
                              nc.tensor.matmul(
                                  sS[:, QT * h:QT * h + QT],
                                  kT8[pr][64 * h:64 * h + 64,
                                          128 * kt:128 * kt + 128],
                                  qT8[64 * h:64 * h + 64,
                                      QT * pr:QT * pr + QT],
                                  start=True, stop=True,
                              )
                          pab = ppool.tile([128, 2 * QT], bf16, tag="pab")
                          pv = pab[:].rearrange("k (h q) -> k h q", h=2)
                          nc.scalar.activation(pv, sv, Exp, scale=0.125)
                          # causal 0/1 mask (bf16 SBUF, DVE 2x rate)
                          mk = (mask_sb[:, QT * kt:QT * kt + QT]
                                [:, None, :].to_broadcast((128, 2, QT)))
                          nc.vector.tensor_tensor(pv, pv, mk, Mult)
                          for h, ctx in ((0, ctxA), (1, ctxB)):
                              hg = 2 * pr + h
                              nc.tensor.matmul(
                                  ctx[:],
                                  vE[kt][:, 65 * hg:65 * hg + 65],
                                  pab[:, QT * h:QT * h + QT],
                                  start=(kt == 0), stop=(kt == NKT - 1),
                              )
                      for h, ctx in ((0, ctxA), (1, ctxB)):
                          rec = rpool.tile([1, QT], f32, tag="rec")
                          nc.vector.reciprocal(rec[:], ctx[64:65, :])
                          rb = rpool.tile([64, QT], f32, tag="rb")
                          nc.gpsimd.partition_broadcast(rb[:], rec[:])
                          nc.vector.tensor_tensor(
                              ctx_sb[64 * h:64 * h + 64,
                                     QT * pr:QT * pr + QT],
                              ctx[0:64, :], rb[:], Mult)

                  # ---- fc_out (full local contraction) ----
                  for ot in range(8):
                      ps = ppsp.tile([128, QT], f32, tag="pp",
                                     name=f"po{ot}_{_it}")
                      for t in range(8):
                          nc.tensor.matmul(
                              ps[:],
                              wo_sb[:, E * t + 128 * ot:
                                    E * t + 128 * ot + 128],
                              ctx_sb[:, QT * t:QT * t + QT],
                              start=(t == 0), stop=(t == 7),
                          )
                      o_t = otp.tile([128, QT], f32, tag="ot")
                      nc.scalar.activation(o_t[:], ps[:], Ident,
                                           bias=bo_sb[:, ot:ot + 1],
                                           scale=1.0)
                      nc.sync.dma_start(
                          out_d[128 * ot:128 * ot + 128, :], o_t[:])

    nc.compile()
    return nc


def _prep_inputs(key, query, value, Wq, Wk, Wv, Wo, bo):
    import ml_dtypes
    bf16 = ml_dtypes.bfloat16
    f32 = np.float32
    WqT = np.ascontiguousarray(Wq.T.astype(f32))
    WkT = np.ascontiguousarray(Wk.T.astype(f32))
    WvT = np.ascontiguousarray(Wv.T.astype(f32))
    WoT = np.ascontiguousarray(Wo.T.astype(f32))

    wv65 = np.zeros((E, H, 65), dtype=f32)
    wv65[:, :, :64] = WvT.reshape(E, H, DK)
    wqkv = np.concatenate(
        [WqT, WkT, wv65.reshape(E, WV65)], axis=1).astype(bf16)
    wqkv = np.ascontiguousarray(wqkv)
    bo_t = np.ascontiguousarray(bo.astype(f32).reshape(8, 128).T)

    xT = {}
    for name, x in (("q", query), ("k", key), ("v", value)):
        for b in range(B):
            xT[(name, b)] = np.ascontiguousarray(x[b].T.astype(bf16))

    q_idx = np.arange(QT)[None, :]
    k_idx = np.arange(128)[:, None]

    in_maps = []
    for c in range(N_CORES):
        b, g = c // 4, c % 4
        mask01 = np.stack([
            (QT * g + q_idx >= 128 * kt + k_idx) for kt in range(NKT)
        ]).astype(bf16)
        in_maps.append({
            "xqTm": np.ascontiguousarray(xT[("q", b)][:, QT * g:QT * g + QT]),
            "xkT": xT[("k", b)],
            "xvT": xT[("v", b)],
            "wqkv": wqkv,
            "wo": WoT.astype(bf16),
            "bo_t": bo_t,
            "mask01": mask01,
        })
    return in_maps


def kernel(key, query, value, Wq, Wk, Wv, Wo, bo, mask, _return_perf=False):
    from concourse.bass_utils import run_bass_kernel_spmd

    if "nc" not in _CACHE:
        _CACHE["nc"] = _build()
    nc = _CACHE["nc"]

    key = np.asarray(key, dtype=np.float32)
    query = np.asarray(query, dtype=np.float32)
    value = np.asarray(value, dtype=np.float32)
    in_maps = _prep_inputs(key, query, value,
                           np.asarray(Wq), np.asarray(Wk), np.asarray(Wv),
                           np.asarray(Wo), np.asarray(bo))

    res = run_bass_kernel_spmd(nc, in_maps, core_ids=list(range(N_CORES)),
                               trace=_return_perf)

    out = np.empty((B, S, E), dtype=np.float32)
    for c in range(N_CORES):
        b, g = c // 4, c % 4
        out[b, QT * g:QT * g + QT, :] = res.results[c]["outT"].T
    if _return_perf:
        return out, res
    return out


# revision 32
# speedup vs baseline: 1.8909x; 1.8909x over previous
"""Trainium2 Bass kernel for nn_MultiHeadAttention (B=2, S=2048, E=1024, H=16).

Sharding (Megatron-style, per hint): 8 cores = 2 batches x 4 head-groups
(4 heads each). Per core:
  - Q/K projections produce Q^T/K^T in [d_k, S] layout (head-pairs packed to
    128 partitions) so scores can be computed transposed: S^T[k, q] with keys
    on partitions. The two heads of a pair run as concurrent PE row-tiles
    (tile_position rows 0/64, auto-derived from base partitions). Softmax
    reduction over k comes out of the attn@V matmul via a fused ones-column
    in V (row 64 of the context PSUM is the softmax denominator).
  - Causality: score tiles fully above the diagonal are skipped, and diagonal
    steps only compute the valid column range (ascending-j order so the first
    step's PSUM `start` covers the full q width); the 128-wide diagonal band
    is masked with min() against a precomputed triangular +-BIG band.
  - Software pipelining: projection matmuls for s-block nt+1 are WOVEN
    between the attention steps of qt=nt (the PE queue is FIFO, so trailing
    emission could never fill the Act-bound exp bubbles).
  - Communication: context chunks are staged to DRAM per (pair, qt) as they
    are normalized, duplicated into shards s and s+4 of an [8, 2, 128, 512]
    buffer (shard s of every rank = its context for token-block s%4). ONE
    8-core AllToAll then hands every rank exactly its own 512-token slice
    from every peer (1.75MB wire vs 3MB for per-group AllGathers; this
    rig's collective path costs ~235us/MB of wire regardless of op count).
    Each core reads its batch group's 4 shards (dynamic roff = 4*batch) and
    computes its 512-row slice of fc_out with the full Wo, with bias
    combine and per-tile output DMA pipelining the tail.
  - Engine placement: Act does exp (the attention bottleneck) and the
    fc_out bias; masks/normalize/copies on DVE (gpsimd cannot read PSUM).
"""

import numpy as np

N_CORES = 8
B, S, E, H = 2, 2048, 1024, 16
DK = E // H  # 64
HPC = H // 4  # 4 heads per core
GD = HPC * DK  # 256 dims per core
QT = 512  # q tile (free dim of score matmuls)
NQT = S // QT  # 4
W65 = HPC * 65  # 260
WQKV = GD + GD + W65  # 772
BIG = np.float32(3.0e38)

_CACHE = {}


def _build(sim1=False, niter=1, bench_accum=False, local_ag=False):
    import concourse.bacc as bacc
    import concourse.bass as bass
    import concourse.mybir as mybir
    import concourse.tile as tile

    f32 = mybir.dt.float32
    bf16 = mybir.dt.bfloat16

    nc = bacc.Bacc("TRN2", target_bir_lowering=False, debug=False,
                   num_devices=1 if sim1 else N_CORES)

    xq_d = nc.dram_tensor("xqT", [E, S], bf16, kind="ExternalInput")
    xk_d = nc.dram_tensor("xkT", [E, S], bf16, kind="ExternalInput")
    xv_d = nc.dram_tensor("xvT", [E, S], bf16, kind="ExternalInput")
    wqkv_d = nc.dram_tensor("wqkv", [E, WQKV], bf16, kind="ExternalInput")
    wo_d = nc.dram_tensor("wo", [E, E], bf16, kind="ExternalInput")
    bo_d = nc.dram_tensor("bo_t", [128, 8], f32, kind="ExternalInput")
    mask_d = nc.dram_tensor("mask128", [128, 128], f32, kind="ExternalInput")
    roff_d = nc.dram_tensor("roff", [1, 1], mybir.dt.uint32, kind="ExternalInput")
    out_d = nc.dram_tensor("outT", [E, QT], f32, kind="ExternalOutput")

    Exp = mybir.ActivationFunctionType.Exp
    Mult = mybir.AluOpType.mult
    Add = mybir.AluOpType.add
    Min = mybir.AluOpType.min

    with tile.TileContext(nc) as tc:
        # shard-group base register (4 * batch) for the post-AllToAll gather
        regs = nc.alloc_registers("roff_reg")
        nc.regs_load(regs, roff_d[0:1, 0:1])
        roff = nc.snap(regs, donate=True, min_val=0, max_val=4)

        with (
            tc.tile_pool(name="const", bufs=1) as constp,
            tc.tile_pool(name="ctxp", bufs=1) as ctxp,
            tc.tile_pool(name="wop", bufs=1) as wop,
        ):
            wo_sb = wop.tile([128, 8 * E], bf16)

            for _it in range(niter):
              ctxn = ctxp.tile([128, 2 * S], bf16, tag="ctxn",
                               name=f"ctxn{_it}")
              with (
                  tc.tile_pool(name=f"qkv{_it}", bufs=1) as qkvp,
                  tc.tile_pool(name=f"xt{_it}", bufs=6) as xtp,
                  tc.tile_pool(name=f"wgt{_it}", bufs=1) as wgtp,
                  tc.tile_pool(name=f"pps{_it}", bufs=2, space="PSUM") as ppsp,
                  tc.tile_pool(name=f"spool{_it}", bufs=2, space="PSUM") as spool,
                  tc.tile_pool(name=f"cpool{_it}", bufs=1, space="PSUM") as cpool,
                  tc.tile_pool(name=f"ppool{_it}", bufs=3) as ppool,
                  tc.tile_pool(name=f"rpool{_it}", bufs=2) as rpool,
                  tc.tile_pool(name=f"gqp{_it}", bufs=1) as gqp,
                  tc.tile_pool(name=f"osb{_it}", bufs=1) as osbp,
                  tc.tile_pool(name=f"otp{_it}", bufs=3) as otp,
                  tc.tile_pool(name=f"dram{_it}", bufs=1, space="DRAM") as dramp,
              ):
                  qT = [qkvp.tile([128, S], bf16, name=f"qT{m}") for m in range(2)]
                  kTt = [qkvp.tile([128, S], bf16, name=f"kT{m}") for m in range(2)]
                  vE = [qkvp.tile([128, W65], bf16, name=f"vE{s}")
                        for s in range(4 * NQT)]

                  wqkv_sb = wgtp.tile([128, 8 * WQKV], bf16)
                  wq_v = wqkv_sb[:].rearrange("p (t m) -> p t m", t=8)
                  # priority order on the Act HWDGE queue: wq, wk, wv, consts
                  for lo, hi in ((0, GD), (GD, 2 * GD), (2 * GD, WQKV)):
                      nc.scalar.dma_start(
                          wq_v[:, :, lo:hi],
                          wqkv_d.ap().rearrange("(t p) m -> p t m", p=128)
                          [:, :, lo:hi],
                      )
                  mask_sb = constp.tile([128, 128], f32, name=f"mk{_it}")
                  bo_sb = constp.tile([128, 8], f32, name=f"bo{_it}")
                  # non-urgent loads via SWDGE (keeps the Act queue short)
                  nc.gpsimd.dma_start(mask_sb[:], mask_d.ap())
                  nc.gpsimd.dma_start(bo_sb[:], bo_d.ap())
                  nc.gpsimd.dma_start(
                      wo_sb[:].rearrange("p (t m) -> p t m", t=8),
                      wo_d.ap().rearrange("(t p) m -> p t m", p=128),
                  )
                  ag_in = []
                  ag_out = []
                  gq_sb = []

                  def wslice(kt, base, width):
                      return wqkv_sb[:, kt * WQKV + base:
                                     kt * WQKV + base + width]

                  def load_chunk(x_d, nt, name, split=False):
                      # s-column chunk: [128, 8(kt), 512] for s-block nt
                      t = xtp.tile([128, 8 * QT], bf16, tag="xt", name=name)
                      tv = t[:].rearrange("p (t q) -> p t q", t=8)
                      sv = (x_d[:, QT * nt:QT * nt + QT]
                            .rearrange("(t p) q -> p t q", p=128))
                      if split:
                          nc.sync.dma_start(tv[:, 0:4], sv[:, 0:4])
                          nc.sync.dma_start(tv[:, 4:8], sv[:, 4:8])
                      else:
                          nc.sync.dma_start(tv, sv)
                      return t

                  def load_chunks(nt):
                      first = nt == 0
                      return (load_chunk(xq_d, nt, f"xq{nt}", split=first),
                              load_chunk(xk_d, nt, f"xk{nt}", split=first),
                              load_chunk(xv_d, nt, f"xv{nt}"))

                  def proj_units(nt, chq, chk, chv):
                      """Micro-units (closures) projecting s-block nt.
                      Each unit is ~2-3 matmuls so it can be woven between
                      attention steps without stalling the Act pipeline."""
                      units = []
                      for wbase, dst, ch in ((0, qT, chq), (GD, kTt, chk)):
                          for m in range(2):
                              box = []
                              for k0 in range(0, 8, 2):
                                  def u(wbase=wbase, dst=dst, ch=ch, m=m,
                                        k0=k0, box=box, nt=nt):
                                      if k0 == 0:
                                          box.append(ppsp.tile(
                                              [128, QT], f32, tag="pp",
                                              name=f"pqk{nt}_{wbase}_{m}"))
                                      ps = box[0]
                                      for kt in (k0, k0 + 1):
                                          nc.tensor.matmul(
                                              ps[:],
                                              wslice(kt, wbase + 128 * m, 128),
                                              ch[:, QT * kt:QT * kt + QT],
                                              start=(kt == 0), stop=(kt == 7),
                                          )
                                      if k0 == 6:
                                          nc.vector.tensor_copy(
                                              dst[m][:, QT * nt:QT * nt + QT],
                                              ps[:])
                                  units.append(u)
                      for sst in range(4):
                          st = 4 * nt + sst
                          box = []
                          for k0 in range(0, 9, 3):
                              def u(sst=sst, st=st, k0=k0, box=box, chv=chv):
                                  if k0 == 0:
                                      box.append(ppsp.tile(
                                          [128, W65], f32, tag="pp",
                                          name=f"psv{st}"))
                                  ps = box[0]
                                  for kt in range(k0, min(k0 + 3, 8)):
                                      nc.tensor.matmul(
                                          ps[:],
                                          chv[:, QT * kt + 128 * sst:
                                              QT * kt + 128 * sst + 128],
                                          wslice(kt, 2 * GD, W65),
                                          start=(kt == 0), stop=(kt == 7),
                                      )
                                  if k0 == 6:
                                      nc.vector.tensor_copy(vE[st][:], ps[:])
                                      # softmax-denominator ones column (the
                                      # wv65 zero column becomes 1.0 here)
                                      nc.gpsimd.memset(
                                          vE[st][:].rearrange(
                                              "p (h c) -> p h c", h=HPC)
                                          [:, :, 64:65], 1.0)
                              units.append(u)
                      return units

                  pending = []

                  def pump(k):
                      for _ in range(min(k, len(pending))):
                          pending.pop(0)()

                  # ========= Interleaved projections + attention =========
                  chunks = load_chunks(0)
                  for u in proj_units(0, *chunks):
                      u()
                  for nt in range(NQT):
                      if nt + 1 < NQT:
                          nxt = load_chunks(nt + 1)
                          pending.extend(proj_units(nt + 1, *nxt))

                      # attention for qt = nt, both head-pairs
                      qt = nt
                      for p in range(2):
                          ctxA = cpool.tile([65, QT], f32, tag="ctxA")
                          ctxB = cpool.tile([65, QT], f32, tag="ctxB")
                          # full steps first (qt>0), then diagonal steps
                          # ascending so si=0 covers the full q width
                          # (PSUM `start` semantics).
                          steps = [(kt, -1, 0) for kt in range(4 * qt)]
                          steps += [(4 * qt + j, j, 128 * j)
                                    for j in range(4)]
                          last = len(steps) - 1
                          for si, (kt, j, off) in enumerate(steps):
                              if qt == NQT - 1 and p == 0 and si == 4 * qt:
                                  pump(len(pending))
                              w = QT - off  # computed q-width
                              sS = spool.tile([128, 2 * QT], f32, tag="s")
                              sv = sS[:].rearrange("k (h q) -> k h q", h=2)
                              for h in range(2):
                                  nc.tensor.matmul(
                                      sS[:, QT * h + off:QT * h + QT],
                                      kTt[p][64 * h:64 * h + 64,
                                             128 * kt:128 * kt + 128],
                                      qT[p][64 * h:64 * h + 64,
                                            QT * qt + off:QT * qt + QT],
                                      start=True, stop=True,
                                  )
                              if j >= 0:
                                  svj = sv[:, :, off:off + 128]
                                  mk = (mask_sb[:, None, :]
                                        .to_broadcast((128, 2, 128)))
                                  nc.vector.tensor_tensor(svj, svj, mk, Min)
                              pab = ppool.tile([128, 2 * QT], bf16,
                                               tag="pab")
                              nc.scalar.activation(
                                  pab[:].rearrange("k (h q) -> k h q", h=2)
                                  [:, :, off:off + w],
                                  sv[:, :, off:off + w], Exp, scale=0.125)
                              for h, ctx in ((0, ctxA), (1, ctxB)):
                                  hg = 2 * p + h
                                  nc.tensor.matmul(
                                      ctx[:, off:off + w],
                                      vE[kt][:, 65 * hg:65 * hg + 65],
                                      pab[:, QT * h + off:QT * h + QT],
                                      start=(si == 0), stop=(si == last),
                                  )
                              pump(1)
                          for h, ctx in ((0, ctxA), (1, ctxB)):
                              rec = rpool.tile([1, QT], f32, tag="rec")
                              nc.vector.reciprocal(rec[:], ctx[64:65, :])
                              rb = rpool.tile([64, QT], f32, tag="rb")
                              nc.gpsimd.partition_broadcast(rb[:], rec[:])
                              nc.vector.tensor_tensor(
                                  ctxn[64 * h:64 * h + 64,
                                       S * p + QT * qt:
                                       S * p + QT * qt + QT],
                                  ctx[0:64, :], rb[:], Mult)

                          # stage this (pair, qt) context chunk to DRAM as
                          # soon as it is normalized (cheap DMA). The
                          # collective path on this rig costs ~235us/MB of
                          # wire regardless of op count, so we use ONE
                          # 8-core AllToAll carrying both batch groups'
                          # exchanges (1.75MB wire) instead of per-group
                          # AllGathers (3MB wire): shard s of every rank
                          # holds that rank's context for token-block s%4,
                          # so rank r receives exactly its own token slice
                          # from every peer.
                          if p == 0 and qt == 0:
                              ag_in.append(dramp.tile(
                                  [8, 2, 128, QT], bf16, name="a2ain"))
                              ag_out.append(dramp.tile(
                                  [8, 2, 128, QT], bf16, name="a2aout"))
                          for s in (qt, qt + 4):
                              nc.sync.dma_start(
                                  ag_in[0][s, p],
                                  ctxn[:, S * p + QT * qt:
                                       S * p + QT * qt + QT])
                          if qt == NQT - 1 and p == 1:
                              if sim1 or local_ag:
                                  for s in range(8):
                                      nc.sync.dma_start(ag_out[0][s],
                                                        ag_in[0][s])
                              else:
                                  nc.gpsimd.collective_compute(
                                      "AllToAll",
                                      mybir.AluOpType.bypass,
                                      replica_groups=[[0, 1, 2, 3,
                                                       4, 5, 6, 7]],
                                      ins=[ag_in[0][:]],
                                      outs=[ag_out[0][:]],
                                  )
                              # pull my batch group's 4 shards (dynamic
                              # abase = 4*batch, via the roff register)
                              for pp in range(2):
                                  g_sb = gqp.tile([128, 4 * QT], bf16,
                                                  tag="gq", name=f"gq{pp}")
                                  nc.gpsimd.dma_start(
                                      g_sb[:].rearrange("p (s q) -> p s q",
                                                        s=4),
                                      ag_out[0][:]
                                      [bass.ds(roff, 4), pp]
                                      .rearrange("s p q -> p s q"))
                                  gq_sb.append(g_sb)

                      # hold back block-3's V units: they are only
                      # needed at qt=3's diagonal steps and fill the
                      # Act-bound bubbles of attention(3)'s full steps
                      hold = 12 if nt == 2 else 0
                      pump(len(pending) - hold)

                  # ===== fc_out pair-0 half: hides under pair-1 AllGather ===
                  o_part = osbp.tile([128, 8 * QT], f32, name="opart")
                  for ot in range(8):
                      ps = ppsp.tile([128, QT], f32, tag="pp", name=f"oa{ot}")
                      for i in range(4):
                          t = 2 * i  # pair-0 e_in block
                          nc.tensor.matmul(
                              ps[:],
                              wo_sb[:, E * t + 128 * ot:E * t + 128 * ot + 128],
                              gq_sb[0][:, QT * i:QT * i + QT],
                              start=(i == 0), stop=(i == 3),
                          )
                      # bias-add on Act (idle after the last exp; DVE is
                      # still draining the pair-1 normalize FIFO here)
                      nc.scalar.activation(
                          o_part[:, QT * ot:QT * ot + QT], ps[:],
                          mybir.ActivationFunctionType.Identity,
                          bias=bo_sb[:, ot:ot + 1], scale=1.0)

                  # ===== tail: pair-1 half + combine + per-tile output DMA ==
                  for ot in range(8):
                      ps = ppsp.tile([128, QT], f32, tag="pp", name=f"ob{ot}")
                      for i in range(4):
                          t = 2 * i + 1  # pair-1 e_in block
                          nc.tensor.matmul(
                              ps[:],
                              wo_sb[:, E * t + 128 * ot:E * t + 128 * ot + 128],
                              gq_sb[1][:, QT * i:QT * i + QT],
                              start=(i == 0), stop=(i == 3),
                          )
                      o_t = otp.tile([128, QT], f32, tag="ot")
                      nc.vector.tensor_tensor(
                          o_t[:], ps[:], o_part[:, QT * ot:QT * ot + QT], Add)
                      if bench_accum:
                          nc.gpsimd.dma_start(
                              out_d[128 * ot:128 * ot + 128, :], o_t[:],
                              accum_op=mybir.AluOpType.add)
                      else:
                          nc.sync.dma_start(
                              out_d[128 * ot:128 * ot + 128, :], o_t[:])

    nc.compile()
    return nc


def _prep_inputs(key, query, value, Wq, Wk, Wv, Wo, bo):
    """Build the 8 per-core input maps (all host-side numpy)."""
    import ml_dtypes
    bf16 = ml_dtypes.bfloat16
    f32 = np.float32
    WqT = np.ascontiguousarray(Wq.T.astype(f32))  # [in, out]
    WkT = np.ascontiguousarray(Wk.T.astype(f32))
    WvT = np.ascontiguousarray(Wv.T.astype(f32))
    WoT = np.ascontiguousarray(Wo.T.astype(f32))  # [e_in, o]

    # wv with a zero column appended per head (65-stride interleave)
    wv65 = np.zeros((E, H, 65), dtype=f32)
    wv65[:, :, :64] = WvT.reshape(E, H, DK)

    bo_t = np.ascontiguousarray(bo.astype(f32).reshape(8, 128).T)

    # triangular band mask for the 128-wide diagonal strip:
    # mask[k, q'] keeps (+BIG) iff q' >= k
    q_idx = np.arange(128)[None, :]
    k_idx = np.arange(128)[:, None]
    mask128 = np.where(q_idx >= k_idx, BIG, -BIG).astype(f32)

    xT = {}
    for name, x in (("q", query), ("k", key), ("v", value)):
        for b in range(B):
            xT[(name, b)] = np.ascontiguousarray(x[b].T.astype(bf16))

    in_maps = []
    for c in range(N_CORES):
        b, g = c // 4, c % 4
        heads = slice(g * GD, (g + 1) * GD)
        wqkv = np.concatenate(
            [WqT[:, heads], WkT[:, heads],
             wv65[:, 4 * g:4 * g + 4, :].reshape(E, W65)],
            axis=1).astype(bf16)
        in_maps.append({
            "xqT": xT[("q", b)],
            "xkT": xT[("k", b)],
            "xvT": xT[("v", b)],
            "wqkv": np.ascontiguousarray(wqkv),
            "wo": WoT.astype(bf16),
            "bo_t": bo_t,
            "mask128": mask128,
            "roff": np.array([[4 * b]], dtype=np.uint32),  # A2A shard base
        })
    return in_maps


def kernel(key, query, value, Wq, Wk, Wv, Wo, bo, mask, _return_perf=False):
    from concourse.bass_utils import run_bass_kernel_spmd

    if "nc" not in _CACHE:
        _CACHE["nc"] = _build()
    nc = _CACHE["nc"]

    key = np.asarray(key, dtype=np.float32)
    query = np.asarray(query, dtype=np.float32)
    value = np.asarray(value, dtype=np.float32)
    in_maps = _prep_inputs(key, query, value,
                           np.asarray(Wq), np.asarray(Wk), np.asarray(Wv),
                           np.asarray(Wo), np.asarray(bo))

    res = run_bass_kernel_spmd(nc, in_maps, core_ids=list(range(N_CORES)),
                               trace=_return_perf)

    out = np.empty((B, S, E), dtype=np.float32)
    for c in range(N_CORES):
        b, g = c // 4, c % 4
        out[b, QT * g:QT * g + QT, :] = res.results[c]["outT"].T
    if _return_perf:
        return out, res
    return out
